# revision 9
# baseline (speedup 1.0000x reference)
"""Trainium2 Bass kernel for nn_AggressiveForgettingController.

Strategy (sharding_hint): shard memory_bank along the slot dimension across
8 NeuronCores. The device computes, per shard, the two S-sized contractions
that dominate the memory traffic:
  sims_raw[b, s] = sum_d l2n(new_content)[b, d] * memory_bank[s, d]   (32 rows)
  conf_raw[j, s] = sum_d conf_w[j, d]          * memory_bank[s, d]   (32 rows)
as one fused 64-output matmul over the shard. The host transposes the shard
to [d, s] layout (the PE contracts over the partition dim), replicates the
64-column weight, runs SPMD over cores 0-7, gathers the [64, S] result, and
finishes the tiny B-sized MLP/control logic in f32 numpy mirroring the
reference semantics exactly.
"""

import os

import numpy as np

S, D, B, H = 131072, 512, 32, 256
NCORES = 8
SSH = S // NCORES           # 16384 slots per core
NSL = 512                   # slots per matmul (PSUM bank limit for f32)
NSLX = int(os.environ.get("KERNEL_NSLX", "2048"))  # slots per DMA x-tile
NGR = NSLX // NSL           # matmul groups per x-tile
NT = SSH // NSLX
CAP_LIMIT, MIN_NOV, STORE_TH, ACT_TH = 0.85, 0.4, 0.35, 0.5
STAB_TARGET = 0.1

MM_DTYPE = os.environ.get("KERNEL_MM_DTYPE", "float32")

_cache = {}
last_results = None


def _install_ntff_hook_shim():
    """The agent image's `antenv` lacks `axon_hooks`; recreate it so
    run_bass_kernel_spmd(trace=True) can capture NTFF profiles through the
    axon PJRT .so. Best-effort — profiling only, never required for results."""
    import sys
    if "antenv.axon_hooks" in sys.modules:
        return
    try:
        import contextlib
        import ctypes
        import types

        so_path = "/opt/axon/libaxon_pjrt.so"
        lib = ctypes.CDLL(so_path)
        if not hasattr(lib, "axon_start_nrt_profile"):
            return
        lib.axon_start_nrt_profile.argtypes = [
            ctypes.POINTER(ctypes.c_int64), ctypes.c_size_t]
        lib.axon_start_nrt_profile.restype = ctypes.c_int64
        lib.axon_stop_nrt_profile.argtypes = [ctypes.c_char_p]
        lib.axon_stop_nrt_profile.restype = ctypes.c_int64

        @contextlib.contextmanager
        def _hook(output_dir, device_ids):
            import jax
            jax.devices()
            if device_ids:
                ids = (ctypes.c_int64 * len(device_ids))(*device_ids)
                rc = lib.axon_start_nrt_profile(ids, len(device_ids))
            else:
                rc = lib.axon_start_nrt_profile(None, 0)
            if rc != 0:
                raise RuntimeError(f"axon_start_nrt_profile rc={rc}")
            try:
                yield
            finally:
                n = lib.axon_stop_nrt_profile(str(output_dir).encode())
                print(f"ntff profile: {n} file(s) written to {output_dir}")

        mod = types.ModuleType("antenv.axon_hooks")
        _holder = [_hook]
        mod.set_axon_ntff_profile_hook = lambda h: _holder.__setitem__(0, h)
        mod.get_axon_ntff_profile_hook = lambda: _holder[0]
        sys.modules["antenv.axon_hooks"] = mod
    except Exception as e:  # pragma: no cover
        print(f"ntff hook shim unavailable: {e}")


def _build_nc():
    import concourse.bacc as bacc
    import concourse.mybir as mybir
    from concourse import tile

    f32 = mybir.dt.float32
    mm_dt = getattr(mybir.dt, MM_DTYPE)

    nc = bacc.Bacc("TRN2", target_bir_lowering=False, debug=False,
                   num_devices=NCORES)

    memT = nc.dram_tensor("memT", [4, 128, SSH], mm_dt, kind="ExternalInput").ap()
    wmat = nc.dram_tensor("wmat", [128, 4, 64], mm_dt, kind="ExternalInput").ap()
    out = nc.dram_tensor("out", [64, SSH], f32, kind="ExternalOutput").ap()

    with tile.TileContext(nc) as tc:
        with (
            tc.tile_pool(name="wp", bufs=1) as wp,
            tc.tile_pool(name="xp", bufs=3) as xp,
            tc.tile_pool(name="op", bufs=3) as op,
            tc.tile_pool(name="pp", bufs=8, space="PSUM") as pp,
        ):
            w_sb = wp.tile([128, 4, 64], mm_dt)
            nc.sync.dma_start(w_sb[:], wmat[:])
            for t in range(NT):
                x = xp.tile([128, 4, NSLX], mm_dt, tag="x")
                for c in range(4):
                    nc.sync.dma_start(x[:, c, :], memT[c, :, t * NSLX:(t + 1) * NSLX])
                o_sb = op.tile([64, NSLX], f32, tag="o")
                for g in range(NGR):
                    sl = slice(g * NSL, (g + 1) * NSL)
                    ps = pp.tile([64, NSL], f32, tag="ps")
                    for c in range(4):
                        nc.tensor.matmul(ps[:], w_sb[:, c, :], x[:, c, sl],
                                         start=(c == 0), stop=(c == 3))
                    nc.vector.tensor_copy(o_sb[:, sl], ps[:])
                nc.sync.dma_start(out[:, t * NSLX:(t + 1) * NSLX], o_sb[:])
    nc.compile()
    return nc


def _device_sims_conf(memory_bank, nhat, conf_w):
    """Returns sims_raw [B, S] (= nhat @ mem.T) and conf_raw [S, 32]
    (= mem @ conf_w.T, pre-bias), both f32, computed on 8 NeuronCores."""
    global last_results
    from concourse.bass_utils import run_bass_kernel_spmd

    if "nc" not in _cache:
        _cache["nc"] = _build_nc()
    nc = _cache["nc"]

    W = np.empty((D, 64), np.float32)
    W[:, :32] = nhat.T
    W[:, 32:] = conf_w.T
    wmat = np.ascontiguousarray(W.reshape(4, 128, 64).transpose(1, 0, 2))

    in_maps = []
    for i in range(NCORES):
        shard = memory_bank[i * SSH:(i + 1) * SSH]
        memT = np.ascontiguousarray(shard.T).reshape(4, 128, SSH)
        in_maps.append({"memT": memT, "wmat": wmat})

    trace = bool(int(os.environ.get("KERNEL_TRACE", "0")))
    if trace:
        _install_ntff_hook_shim()
    res = run_bass_kernel_spmd(nc, in_maps, list(range(NCORES)), trace=trace)
    last_results = res

    outs = [r["out"] for r in res.results]
    sims_raw = np.concatenate([o[:32] for o in outs], axis=1)
    conf_raw = np.concatenate([o[32:] for o in outs], axis=1).T
    return sims_raw, np.ascontiguousarray(conf_raw)


def _lin(x, w, b):
    return x @ w.T + b


def _ln(x, g, b):
    m = x.mean(-1, keepdims=True, dtype=np.float32)
    v = ((x - m) ** 2).mean(-1, keepdims=True, dtype=np.float32)
    return (x - m) / np.sqrt(v + np.float32(1e-5)) * g + b


def _sigmoid(x):
    with np.errstate(over="ignore", under="ignore"):
        return np.float32(1.0) / (np.float32(1.0) + np.exp(-x))


def _relu(x):
    return np.maximum(x, np.float32(0.0))


def kernel(new_content, query, memory_bank, access_times,
           rel_w1, rel_b1, rel_g, rel_bb, rel_w2, rel_b2,
           nov_w, nov_b, gate_w, gate_b,
           lru_w, lru_b, conf_w, conf_b, eg_w, eg_b,
           dd_w1, dd_b1, dd_w2, dd_b2,
           ds_w1, ds_b1, ds_w2, ds_b2,
           decay_rate, memory_change_ema, current_step):
    f32 = np.float32
    new_content = np.asarray(new_content, f32)
    query = np.asarray(query, f32)
    memory_bank = np.ascontiguousarray(np.asarray(memory_bank, f32))
    access_times = np.asarray(access_times, f32)
    step = f32(current_step)
    mem0 = memory_bank

    # ---- row norms of mem0 (host; the device already reads mem once) ----
    sumsq = np.einsum("sd,sd->s", mem0, mem0)
    norms = np.sqrt(sumsq)
    capacity = f32((norms > f32(ACT_TH)).sum()) / f32(S)

    # ---- device: fused sims/conf matmuls over the slot dimension ----
    nc_norm = np.linalg.norm(new_content, axis=-1, keepdims=True)
    nhat = new_content / np.maximum(nc_norm, f32(1e-12))
    sims_raw, conf_raw = _device_sims_conf(memory_bank, nhat, np.asarray(conf_w, f32))
    sims = sims_raw / np.maximum(norms, f32(1e-12))[None, :]          # [B, S]

    # ---- store gating ----
    combined = np.concatenate([new_content, query], axis=-1)
    rel = _relu(_lin(_relu(_ln(_lin(combined, rel_w1, rel_b1), rel_g, rel_bb)),
                     rel_w2, rel_b2))
    max_sim = sims.max(axis=-1, keepdims=True)                        # [B, 1]
    novelty = (f32(1.0) - max_sim) / f32(2.0)
    nov_feat = _sigmoid(_lin(new_content, nov_w, nov_b))
    store_score = _sigmoid(_lin(rel + nov_feat, gate_w, gate_b))      # [B, 1]
    store_mean = f32(store_score.mean(dtype=np.float32))
    novelty_mean = f32(novelty.mean(dtype=np.float32))
    base_store = store_mean > f32(STORE_TH)
    novelty_ok = novelty_mean > f32(MIN_NOV)
    should_store = bool(base_store and novelty_ok)
    over_cap = bool(capacity > f32(CAP_LIMIT))

    # ---- emergency erase (applied only when over capacity) ----
    age = np.maximum(step - access_times, f32(0.0))
    erase_sc0 = age / (age.max() + f32(1e-6)) + (f32(1.0) - _sigmoid(norms))
    victim = int(np.argmax(erase_sc0))

    mem = mem0.copy()
    at = access_times.copy()
    if over_cap:
        mem[victim] = f32(0.0)
        at[victim] = f32(-99999.0)

    # ---- learned erase scores ----
    lru = _relu(_lin(((step - at) / f32(1000.0))[:, None], lru_w, lru_b))  # [S, 32]
    if over_cap:
        conf_raw[victim] = f32(0.0)
    conf = _sigmoid(conf_raw + conf_b)                                     # [S, 32]
    eg_w = np.asarray(eg_w, f32)
    erase_logit = lru @ eg_w[:, :32].T + conf @ eg_w[:, 32:].T + eg_b
    erase_scores = _sigmoid(erase_logit)[:, 0]

    # ---- conflict detection + drift (sequential masked scatter) ----
    sims2 = sims
    if over_cap:
        sims2 = sims.copy()
        sims2[:, victim] = f32(0.0)
    # jax.lax.top_k: values descending, ties -> lower index first
    cand = np.argpartition(-sims2, 8, axis=1)[:, :8]
    topi = np.empty((B, 3), np.int64)
    topv = np.empty((B, 3), f32)
    for b in range(B):
        idx = cand[b]
        # jax tie-break: equal values keep ascending index order
        order = sorted(idx, key=lambda i: (-sims2[b, i], i))[:3]
        topi[b] = order
        topv[b] = sims2[b, order]
    conflict_mask = (topv > f32(0.7)) & (topv < f32(0.99))
    old = mem[topi]                                                   # [B, 3, D]
    newb = np.broadcast_to(new_content[:, None, :], (B, 3, D))
    pair = np.concatenate([np.ascontiguousarray(newb), old], axis=-1)
    prob = _sigmoid(_lin(_relu(_lin(pair, dd_w1, dd_b1)), dd_w2, dd_b2))[..., 0]
    strength = _sigmoid(_lin(_relu(_lin(pair, ds_w1, ds_b1)), ds_w2, ds_b2))
    avg = (newb + old) / f32(2.0)
    cand_new = (f32(1.0) - strength) * newb + strength * avg
    cand_old = (f32(1.0) - strength) * old + strength * avg
    active = (conflict_mask & (prob > f32(0.5))).reshape(-1)
    fi = topi.reshape(-1)
    fn = cand_new.reshape(B * 3, D)
    fo = cand_old.reshape(B * 3, D)
    drifted_new = new_content.copy()
    for i in range(B * 3):
        if active[i]:
            drifted_new[i // 3] = fn[i]
            mem[fi[i]] = fo[i]

    # ---- conditional store ----
    write_idx = victim if over_cap else int(np.argmax(erase_scores))
    if should_store:
        mem[write_idx] = drifted_new[0]
        at[write_idx] = step

    # ---- stability / plasticity decay update ----
    changed = {victim, write_idx} | {int(fi[i]) for i in range(B * 3) if active[i]}
    total = np.float64(0.0)
    for r in changed:
        total += np.abs(mem[r].astype(np.float64) - mem0[r].astype(np.float64)).sum()
    change = f32(total / (S * D))
    memory_change_ema = np.asarray(memory_change_ema, f32)
    ema_new = f32(0.9) * memory_change_ema + f32(0.1) * change        # [1]
    adjustment = f32(-0.005) if float(ema_new.sum()) > STAB_TARGET else f32(0.005)
    decay_new = np.clip(f32(decay_rate) + adjustment, f32(0.01), f32(0.2))
    relevance = f32(_sigmoid(max_sim).mean(dtype=np.float32))

    return (mem, at, erase_scores, store_mean, novelty_mean, relevance,
            capacity, f32(conflict_mask.sum()), f32(decay_new),
            ema_new.astype(f32))


# revision 11
# speedup vs baseline: 1.2398x; 1.2398x over previous
"""Trainium2 Bass kernel for nn_AggressiveForgettingController.

Strategy (sharding_hint): shard memory_bank along the slot dimension across
8 NeuronCores. The device computes, per shard, the two S-sized contractions
that dominate the memory traffic:
  sims_raw[b, s] = sum_d l2n(new_content)[b, d] * memory_bank[s, d]   (32 rows)
  conf_raw[j, s] = sum_d conf_w[j, d]          * memory_bank[s, d]   (32 rows)
as one fused 64-output matmul over the shard. The host transposes the shard
to [d, s] layout (the PE contracts over the partition dim), replicates the
64-column weight, runs SPMD over cores 0-7, gathers the [64, S] result, and
finishes the tiny B-sized MLP/control logic in f32 numpy mirroring the
reference semantics exactly.
"""

import os

import numpy as np

S, D, B, H = 131072, 512, 32, 256
NCORES = 8
SSH = S // NCORES           # 16384 slots per core
NSL = 512                   # slots per matmul (PSUM bank limit for f32)
NSLX = int(os.environ.get("KERNEL_NSLX", "2048"))  # slots per DMA x-tile
NGR = NSLX // NSL           # matmul groups per x-tile
NT = SSH // NSLX
CAP_LIMIT, MIN_NOV, STORE_TH, ACT_TH = 0.85, 0.4, 0.35, 0.5
STAB_TARGET = 0.1

MM_DTYPE = os.environ.get("KERNEL_MM_DTYPE", "float32")

_cache = {}
last_results = None


def _install_ntff_hook_shim():
    """The agent image's `antenv` lacks `axon_hooks`; recreate it so
    run_bass_kernel_spmd(trace=True) can capture NTFF profiles through the
    axon PJRT .so. Best-effort — profiling only, never required for results."""
    import sys
    if "antenv.axon_hooks" in sys.modules:
        return
    try:
        import contextlib
        import ctypes
        import types

        so_path = "/opt/axon/libaxon_pjrt.so"
        lib = ctypes.CDLL(so_path)
        if not hasattr(lib, "axon_start_nrt_profile"):
            return
        lib.axon_start_nrt_profile.argtypes = [
            ctypes.POINTER(ctypes.c_int64), ctypes.c_size_t]
        lib.axon_start_nrt_profile.restype = ctypes.c_int64
        lib.axon_stop_nrt_profile.argtypes = [ctypes.c_char_p]
        lib.axon_stop_nrt_profile.restype = ctypes.c_int64

        @contextlib.contextmanager
        def _hook(output_dir, device_ids):
            import jax
            jax.devices()
            if device_ids:
                ids = (ctypes.c_int64 * len(device_ids))(*device_ids)
                rc = lib.axon_start_nrt_profile(ids, len(device_ids))
            else:
                rc = lib.axon_start_nrt_profile(None, 0)
            if rc != 0:
                raise RuntimeError(f"axon_start_nrt_profile rc={rc}")
            try:
                yield
            finally:
                n = lib.axon_stop_nrt_profile(str(output_dir).encode())
                print(f"ntff profile: {n} file(s) written to {output_dir}")

        mod = types.ModuleType("antenv.axon_hooks")
        _holder = [_hook]
        mod.set_axon_ntff_profile_hook = lambda h: _holder.__setitem__(0, h)
        mod.get_axon_ntff_profile_hook = lambda: _holder[0]
        sys.modules["antenv.axon_hooks"] = mod
    except Exception as e:  # pragma: no cover
        print(f"ntff hook shim unavailable: {e}")


def _build_nc():
    import concourse.bacc as bacc
    import concourse.mybir as mybir
    from concourse import tile

    f32 = mybir.dt.float32
    mm_dt = getattr(mybir.dt, MM_DTYPE)

    nc = bacc.Bacc("TRN2", target_bir_lowering=False, debug=False,
                   num_devices=NCORES)

    memT = nc.dram_tensor("memT", [4, 128, SSH], mm_dt, kind="ExternalInput").ap()
    wmat = nc.dram_tensor("wmat", [128, 4, 64], mm_dt, kind="ExternalInput").ap()
    out = nc.dram_tensor("out", [64, SSH], f32, kind="ExternalOutput").ap()

    with tile.TileContext(nc) as tc:
        with (
            tc.tile_pool(name="wp", bufs=1) as wp,
            tc.tile_pool(name="xp", bufs=(2 if NSLX >= 4096 else 3)) as xp,
            tc.tile_pool(name="op", bufs=(2 if NSLX >= 4096 else 3)) as op,
            tc.tile_pool(name="pp", bufs=8, space="PSUM") as pp,
        ):
            w_sb = wp.tile([128, 4, 64], mm_dt)
            nc.sync.dma_start(w_sb[:], wmat[:])
            for t in range(NT):
                x = xp.tile([128, 4, NSLX], mm_dt, tag="x")
                for c in range(4):
                    eng = nc.sync if c % 2 == 0 else nc.scalar
                    eng.dma_start(x[:, c, :], memT[c, :, t * NSLX:(t + 1) * NSLX])
                o_sb = op.tile([64, NSLX], f32, tag="o")
                for g in range(NGR):
                    sl = slice(g * NSL, (g + 1) * NSL)
                    ps = pp.tile([64, NSL], f32, tag="ps")
                    for c in range(4):
                        nc.tensor.matmul(ps[:], w_sb[:, c, :], x[:, c, sl],
                                         start=(c == 0), stop=(c == 3))
                    nc.vector.tensor_copy(o_sb[:, sl], ps[:])
                nc.scalar.dma_start(out[:, t * NSLX:(t + 1) * NSLX], o_sb[:])
    nc.compile()
    return nc


def _device_sims_conf(memory_bank, nhat, conf_w):
    """Returns sims_raw [B, S] (= nhat @ mem.T) and conf_raw [S, 32]
    (= mem @ conf_w.T, pre-bias), both f32, computed on 8 NeuronCores."""
    global last_results
    from concourse.bass_utils import run_bass_kernel_spmd

    if "nc" not in _cache:
        _cache["nc"] = _build_nc()
    nc = _cache["nc"]

    W = np.empty((D, 64), np.float32)
    W[:, :32] = nhat.T
    W[:, 32:] = conf_w.T
    wmat = np.ascontiguousarray(W.reshape(4, 128, 64).transpose(1, 0, 2))

    in_maps = []
    for i in range(NCORES):
        shard = memory_bank[i * SSH:(i + 1) * SSH]
        memT = np.ascontiguousarray(shard.T).reshape(4, 128, SSH)
        in_maps.append({"memT": memT, "wmat": wmat})

    trace = bool(int(os.environ.get("KERNEL_TRACE", "0")))
    if trace:
        _install_ntff_hook_shim()
    res = run_bass_kernel_spmd(nc, in_maps, list(range(NCORES)), trace=trace)
    last_results = res

    outs = [r["out"] for r in res.results]
    sims_raw = np.concatenate([o[:32] for o in outs], axis=1)
    conf_raw = np.concatenate([o[32:] for o in outs], axis=1).T
    return sims_raw, np.ascontiguousarray(conf_raw)


def _lin(x, w, b):
    return x @ w.T + b


def _ln(x, g, b):
    m = x.mean(-1, keepdims=True, dtype=np.float32)
    v = ((x - m) ** 2).mean(-1, keepdims=True, dtype=np.float32)
    return (x - m) / np.sqrt(v + np.float32(1e-5)) * g + b


def _sigmoid(x):
    with np.errstate(over="ignore", under="ignore"):
        return np.float32(1.0) / (np.float32(1.0) + np.exp(-x))


def _relu(x):
    return np.maximum(x, np.float32(0.0))


def kernel(new_content, query, memory_bank, access_times,
           rel_w1, rel_b1, rel_g, rel_bb, rel_w2, rel_b2,
           nov_w, nov_b, gate_w, gate_b,
           lru_w, lru_b, conf_w, conf_b, eg_w, eg_b,
           dd_w1, dd_b1, dd_w2, dd_b2,
           ds_w1, ds_b1, ds_w2, ds_b2,
           decay_rate, memory_change_ema, current_step):
    f32 = np.float32
    new_content = np.asarray(new_content, f32)
    query = np.asarray(query, f32)
    memory_bank = np.ascontiguousarray(np.asarray(memory_bank, f32))
    access_times = np.asarray(access_times, f32)
    step = f32(current_step)
    mem0 = memory_bank

    # ---- row norms of mem0 (host; the device already reads mem once) ----
    sumsq = np.einsum("sd,sd->s", mem0, mem0)
    norms = np.sqrt(sumsq)
    capacity = f32((norms > f32(ACT_TH)).sum()) / f32(S)

    # ---- device: fused sims/conf matmuls over the slot dimension ----
    nc_norm = np.linalg.norm(new_content, axis=-1, keepdims=True)
    nhat = new_content / np.maximum(nc_norm, f32(1e-12))
    sims_raw, conf_raw = _device_sims_conf(memory_bank, nhat, np.asarray(conf_w, f32))
    sims = sims_raw / np.maximum(norms, f32(1e-12))[None, :]          # [B, S]

    # ---- store gating ----
    combined = np.concatenate([new_content, query], axis=-1)
    rel = _relu(_lin(_relu(_ln(_lin(combined, rel_w1, rel_b1), rel_g, rel_bb)),
                     rel_w2, rel_b2))
    max_sim = sims.max(axis=-1, keepdims=True)                        # [B, 1]
    novelty = (f32(1.0) - max_sim) / f32(2.0)
    nov_feat = _sigmoid(_lin(new_content, nov_w, nov_b))
    store_score = _sigmoid(_lin(rel + nov_feat, gate_w, gate_b))      # [B, 1]
    store_mean = f32(store_score.mean(dtype=np.float32))
    novelty_mean = f32(novelty.mean(dtype=np.float32))
    base_store = store_mean > f32(STORE_TH)
    novelty_ok = novelty_mean > f32(MIN_NOV)
    should_store = bool(base_store and novelty_ok)
    over_cap = bool(capacity > f32(CAP_LIMIT))

    # ---- emergency erase (applied only when over capacity) ----
    age = np.maximum(step - access_times, f32(0.0))
    erase_sc0 = age / (age.max() + f32(1e-6)) + (f32(1.0) - _sigmoid(norms))
    victim = int(np.argmax(erase_sc0))

    mem = mem0.copy()
    at = access_times.copy()
    if over_cap:
        mem[victim] = f32(0.0)
        at[victim] = f32(-99999.0)

    # ---- learned erase scores ----
    lru = _relu(_lin(((step - at) / f32(1000.0))[:, None], lru_w, lru_b))  # [S, 32]
    if over_cap:
        conf_raw[victim] = f32(0.0)
    conf = _sigmoid(conf_raw + conf_b)                                     # [S, 32]
    eg_w = np.asarray(eg_w, f32)
    erase_logit = lru @ eg_w[:, :32].T + conf @ eg_w[:, 32:].T + eg_b
    erase_scores = _sigmoid(erase_logit)[:, 0]

    # ---- conflict detection + drift (sequential masked scatter) ----
    sims2 = sims
    if over_cap:
        sims2 = sims.copy()
        sims2[:, victim] = f32(0.0)
    # jax.lax.top_k: values descending, ties -> lower index first
    cand = np.argpartition(-sims2, 8, axis=1)[:, :8]
    topi = np.empty((B, 3), np.int64)
    topv = np.empty((B, 3), f32)
    for b in range(B):
        idx = cand[b]
        # jax tie-break: equal values keep ascending index order
        order = sorted(idx, key=lambda i: (-sims2[b, i], i))[:3]
        topi[b] = order
        topv[b] = sims2[b, order]
    conflict_mask = (topv > f32(0.7)) & (topv < f32(0.99))
    old = mem[topi]                                                   # [B, 3, D]
    newb = np.broadcast_to(new_content[:, None, :], (B, 3, D))
    pair = np.concatenate([np.ascontiguousarray(newb), old], axis=-1)
    prob = _sigmoid(_lin(_relu(_lin(pair, dd_w1, dd_b1)), dd_w2, dd_b2))[..., 0]
    strength = _sigmoid(_lin(_relu(_lin(pair, ds_w1, ds_b1)), ds_w2, ds_b2))
    avg = (newb + old) / f32(2.0)
    cand_new = (f32(1.0) - strength) * newb + strength * avg
    cand_old = (f32(1.0) - strength) * old + strength * avg
    active = (conflict_mask & (prob > f32(0.5))).reshape(-1)
    fi = topi.reshape(-1)
    fn = cand_new.reshape(B * 3, D)
    fo = cand_old.reshape(B * 3, D)
    drifted_new = new_content.copy()
    for i in range(B * 3):
        if active[i]:
            drifted_new[i // 3] = fn[i]
            mem[fi[i]] = fo[i]

    # ---- conditional store ----
    write_idx = victim if over_cap else int(np.argmax(erase_scores))
    if should_store:
        mem[write_idx] = drifted_new[0]
        at[write_idx] = step

    # ---- stability / plasticity decay update ----
    changed = {victim, write_idx} | {int(fi[i]) for i in range(B * 3) if active[i]}
    total = np.float64(0.0)
    for r in changed:
        total += np.abs(mem[r].astype(np.float64) - mem0[r].astype(np.float64)).sum()
    change = f32(total / (S * D))
    memory_change_ema = np.asarray(memory_change_ema, f32)
    ema_new = f32(0.9) * memory_change_ema + f32(0.1) * change        # [1]
    adjustment = f32(-0.005) if float(ema_new.sum()) > STAB_TARGET else f32(0.005)
    decay_new = np.clip(f32(decay_rate) + adjustment, f32(0.01), f32(0.2))
    relevance = f32(_sigmoid(max_sim).mean(dtype=np.float32))

    return (mem, at, erase_scores, store_mean, novelty_mean, relevance,
            capacity, f32(conflict_mask.sum()), f32(decay_new),
            ema_new.astype(f32))
